# revision 23
# baseline (speedup 1.0000x reference)
"""Additive (Bahdanau) attention on Trainium2, 8 NeuronCores.

Polynomial reformulation: tanh(x) on the needed range is replaced by an odd
degree-13 polynomial, and the (B,Q,K,H) feature tensor is never materialized:

  scores[q,w] = sum_h wv[h] * tanh(qp[h,q] + kp[h,w])
             ~= sum_{j+m=n odd} c_n*C(n,j) * sum_h (wv*qp^j)[h,q] * (kp^m)[h,w]

i.e. a sum of 56 (j,m) PE matmuls of bf16 power matrices, replacing ~64us of
ACT tanh + ~43us of DVE broadcast-adds per core with ~20us of dense PE MULTs.
Power chains are built on DVE (bf16); the per-pair coefficient-scaled K-power
tiles are split between DVE and the otherwise idle ACT engine.

Work-balanced sharding as before: only valid key columns (k < valid_len) are
computed; the host deals 32-key single-batch chunks across cores.  Chunks on
one core are grouped by batch ("groups"); the Q-side power chains are built
per group, K-side chains are shared.  Each core computes per-chunk partial
unnormalized outputs sum_w e[w,:]*V[w,:] plus the denominator sum_w e[w,:]
via a ones-column appended to V; the host sums partials per batch and
normalizes.

Per core: scores accumulate in PSUM [q=128, W] per q-half; mask-add (DVE) +
exp (ACT) -> e[q,W] f32; per chunk: PE transpose -> eT bf16, AV matmul
(eT[32,128] x [V|1][32,257]) -> partial out, DMA'd straight from PSUM.

Compiled variants are cached by the tuple of per-core group sizes.
"""

import math

import numpy as np

import concourse.bass as bass
import concourse.mybir as mybir
import concourse.tile as tile
from concourse import bacc
from concourse.bass_utils import run_bass_kernel_spmd

B, Q, K, H, D, DV = 8, 256, 256, 256, 256, 256
N_CORES = 8
F32 = mybir.dt.float32
BF16 = mybir.dt.bfloat16
AF = mybir.ActivationFunctionType
KC = 32  # key columns per chunk (single batch per chunk)
DEG = 11
NS = (1, 3, 5, 7, 9, 11)
# weighted LS fit of tanh on [-5.3, 5.3], gaussian(std 0.813)+1e-3 floor
C_POLY = {1: 0.98844822, 3: -0.27666028, 5: 0.05801431, 7: -0.00598764,
          9: 0.00026649, 11: -4.14e-06}


def _mlist(j):
    """m's paired with power j: j+m odd, j+m <= DEG."""
    return [m for m in range(DEG - j + 1) if (j + m) % 2 == 1]


def _coeff(j, m):
    return float(C_POLY[j + m] * math.comb(j + m, j))


def build_nc(gsizes):
    gsizes = tuple(gsizes)
    nchunks = sum(gsizes)
    G = len(gsizes)
    assert G == 2 and gsizes[0] == gsizes[1], "uniform two-group structure"
    WS = KC * gsizes[0]  # per-group slot width
    W = KC * nchunks
    VCB = -(-nchunks // 4)  # chunks packed 4 per 128 partitions
    # bf16 packed input columns
    QT_OFF = 0                       # G * 2 d-blocks * 256 (queriesT)
    KT_OFF = QT_OFF + G * 512        # 2 d-blocks * W (keysT cols)
    WQ_OFF = KT_OFF + 2 * W          # 4 x 128 (dp,hp blocks)
    WK_OFF = WQ_OFF + 512
    QP0_OFF = WK_OFF + 512           # 2 hp x 256 (wv broadcast)
    V_OFF = QP0_OFF + 512            # VCB x 257 ([V|1] rows)
    NBF = V_OFF + VCB * 257
    # f32 packed input columns: mask (W) | identity (128)
    NF = W + 128

    nc = bacc.Bacc("TRN2", target_bir_lowering=False,
                   name="paddattn" + "_".join(str(s) for s in gsizes))
    d_bf = nc.dram_tensor("in_bf", [128, NBF], BF16, kind="ExternalInput")
    d_f = nc.dram_tensor("in_f32", [128, NF], F32, kind="ExternalInput")
    d_outp = nc.dram_tensor("outp", [nchunks * 2, 128, 257], F32,
                            kind="ExternalOutput")

    # strip layout: per hp one big [128, 56*W] bf16 tensor holding the
    # coefficient-scaled K-power tiles, grouped contiguously per j (m-minor)
    jlens = [len(_mlist(j)) for j in range(DEG + 1)]
    joff = [0] * (DEG + 1)
    for j in range(1, DEG + 1):
        joff[j] = joff[j - 1] + jlens[j - 1]
    NPAIR = sum(jlens)

    with tile.TileContext(nc) as tc:
        with (
            tc.tile_pool(name="sb", bufs=1) as sb,
            tc.tile_pool(name="ps_s", bufs=1, space=bass.MemorySpace.PSUM) as ps_s,
        ):
            inbf = sb.tile([128, NBF], BF16, tag="inbf")
            # priority DMA: proj operands first, then QP0/Vones
            nc.sync.dma_start(inbf[:, :QP0_OFF], d_bf[:, :QP0_OFF])
            nc.sync.dma_start(inbf[:, QP0_OFF:], d_bf[:, QP0_OFF:])
            inf = sb.tile([128, NF], F32, tag="inf")
            nc.scalar.dma_start(inf[:], d_f[:])

            qTT = [[inbf[:, QT_OFF + (g * 2 + dp) * 256:QT_OFF + (g * 2 + dp + 1) * 256]
                    for dp in range(2)] for g in range(G)]
            kTT = [inbf[:, KT_OFF + dp * W:KT_OFF + (dp + 1) * W] for dp in range(2)]
            wq = [[inbf[:, WQ_OFF + (dp * 2 + hp) * 128:WQ_OFF + (dp * 2 + hp + 1) * 128]
                   for hp in range(2)] for dp in range(2)]
            wk = [[inbf[:, WK_OFF + (dp * 2 + hp) * 128:WK_OFF + (dp * 2 + hp + 1) * 128]
                   for hp in range(2)] for dp in range(2)]
            qp0 = [inbf[:, QP0_OFF + hp * 256:QP0_OFF + (hp + 1) * 256] for hp in range(2)]
            vones = inbf[:, V_OFF:V_OFF + VCB * 257]
            mask_sb = inf[:, 0:W]
            ident = inf[:, W:W + 128]

            # ---- projections (PE); kproj first (K ladder is latency-critical)
            qT = [[sb.tile([128, 256], BF16, tag=f"qT{g}_{hp}", name=f"qT{g}_{hp}")
                   for hp in range(2)] for g in range(G)]
            KP = [[None, None] for _ in range(DEG + 1)]  # KP[m][hp] plain powers
            for hp in range(2):
                KP[1][hp] = sb.tile([128, W], BF16, tag=f"KP1_{hp}", name=f"KP1_{hp}")
            with tc.tile_pool(name="ps_p", bufs=2, space=bass.MemorySpace.PSUM) as ps_p:
                for hp in range(2):
                    pk = ps_p.tile([128, W], F32, tag="proj", name=f"pk{hp}")
                    for dp in range(2):
                        nc.tensor.matmul(pk[:], wk[dp][hp], kTT[dp],
                                         start=(dp == 0), stop=(dp == 1))
                    nc.scalar.copy(KP[1][hp][:], pk[:])
                for g in range(G):
                    for hp in range(2):
                        pq = ps_p.tile([128, 256], F32, tag="proj", name=f"pq{g}_{hp}")
                        for dp in range(2):
                            nc.tensor.matmul(pq[:], wq[dp][hp], qTT[g][dp],
                                             start=(dp == 0), stop=(dp == 1))
                        nc.vector.tensor_copy(qT[g][hp][:], pq[:])

                # ---- K power x^2-ladder (DVE, deep powers fast), then scales;
                # odd powers first: strip_0 (j=0, the first wave) needs them all.
                # strip layout per hp: [g][pair-slot][WS] so the fused matmul's
                # moving operand per (j, g) is a contiguous [128, len*WS] slice.
                strips = [sb.tile([128, 2 * NPAIR * WS], BF16, tag=f"strip{hp}",
                                  name=f"strip{hp}") for hp in range(2)]

                def ks_slice(j, m, hp, g):
                    mi = _mlist(j).index(m)
                    o = (g * NPAIR + joff[j] + mi) * WS
                    return strips[hp][:, o:o + WS]

                for hp in range(2):
                    t = sb.tile([128, W], BF16, tag=f"KP2_{hp}", name=f"KP2_{hp}")
                    nc.vector.tensor_mul(t[:], KP[1][hp][:], KP[1][hp][:])
                    KP[2][hp] = t
                for m in range(3, DEG + 1, 2):  # odd ladder
                    for hp in range(2):
                        t = sb.tile([128, W], BF16, tag=f"KP{m}_{hp}", name=f"KP{m}_{hp}")
                        nc.vector.tensor_mul(t[:], KP[m - 2][hp][:], KP[2][hp][:])
                        KP[m][hp] = t
                # scale tiles for j=0 (all odd m) with ACT/GPSIMD help
                sc_alt = 0

                def scale(j, m, hp):
                    nonlocal sc_alt
                    c = _coeff(j, m)
                    for g in range(2):
                        dst = ks_slice(j, m, hp, g)
                        if m == 0:
                            nc.gpsimd.memset(dst, c)
                            continue
                        src = KP[m][hp][:, g * WS:(g + 1) * WS]
                        r = sc_alt % 5
                        sc_alt += 1
                        if r in (0, 2):  # ~40% on the otherwise idle ACT
                            nc.scalar.mul(dst, src, c)
                        else:
                            nc.vector.tensor_scalar_mul(dst, src, c)

                for m in _mlist(0):
                    for hp in range(2):
                        scale(0, m, hp)
                for m in range(4, DEG + 1, 2):  # even ladder
                    for hp in range(2):
                        t = sb.tile([128, W], BF16, tag=f"KP{m}_{hp}", name=f"KP{m}_{hp}")
                        nc.vector.tensor_mul(t[:], KP[m - 2][hp][:], KP[2][hp][:])
                        KP[m][hp] = t
                for m in _mlist(1):
                    for hp in range(2):
                        scale(1, m, hp)

                # ---- Q power chains (DVE) interleaved with the per-j scale
                # batches, matching the ascending-wave consumption order
                QP = [[[None, None] for _ in range(DEG + 1)] for _ in range(G)]
                for g in range(G):
                    for hp in range(2):
                        QP[g][0][hp] = qp0[hp]

                def qp_level(j):
                    for g in range(G):
                        for hp in range(2):
                            t = sb.tile([128, 256], BF16, tag=f"QP{g}_{j}_{hp}",
                                        name=f"QP{g}_{j}_{hp}")
                            nc.vector.tensor_mul(t[:], QP[g][j - 1][hp][:],
                                                 qT[g][hp][:])
                            QP[g][j][hp] = t

                qp_level(1)
                for j in range(2, DEG + 1):
                    for m in _mlist(j):
                        for hp in range(2):
                            scale(j, m, hp)
                    qp_level(j)

            # ---- fused score matmuls: one MM per (wave j, qt, g, hp) with
            # rhs = whole j-strip slice, out broadcast over m (PSUM accumulates)
            s_ps = [ps_s.tile([128, W], F32, tag=f"s{qt}", name=f"s{qt}")
                    for qt in range(2)]
            goff = []
            o = 0
            for g in range(G):
                goff.append(o)
                o += KC * gsizes[g]
            e_f = [sb.tile([128, W], F32, tag=f"e{qt}", name=f"e{qt}")
                   for qt in range(2)]
            eT = sb.tile([128, VCB * 256], BF16, tag="eT")
            out_sb = sb.tile([128, 2 * nchunks * 257], F32, tag="out_sb")
            dma_rr = [nc.sync, nc.gpsimd, nc.scalar]

            with tc.tile_pool(name="ps_t", bufs=3, space=bass.MemorySpace.PSUM) as ps_t:

                def tail(qt, g):
                    lo, wg = goff[g], KC * gsizes[g]
                    nc.vector.tensor_add(e_f[qt][:, lo:lo + wg],
                                         s_ps[qt][:, lo:lo + wg],
                                         mask_sb[:, lo:lo + wg])
                    nc.scalar.activation(e_f[qt][:, lo:lo + wg],
                                         e_f[qt][:, lo:lo + wg], AF.Exp)
                    c0 = lo // KC
                    for c in range(c0, c0 + gsizes[g]):
                        r, cb = KC * (c % 4), c // 4
                        tx = ps_t.tile([32, 128], F32, tag="tx")
                        nc.tensor.transpose(tx[:], e_f[qt][:, KC * c:KC * (c + 1)],
                                            ident)
                        nc.vector.tensor_copy(
                            eT[r:r + KC, cb * 256 + qt * 128:cb * 256 + (qt + 1) * 128],
                            tx[:])
                        av = ps_t.tile([128, 257], F32, tag="av")
                        nc.tensor.matmul(
                            av[:],
                            eT[r:r + KC, cb * 256 + qt * 128:cb * 256 + (qt + 1) * 128],
                            vones[r:r + KC, cb * 257:(cb + 1) * 257],
                            start=True, stop=True, tile_position=(r, 0))
                        o = (2 * c + qt) * 257
                        nc.vector.tensor_copy(out_sb[:, o:o + 257], av[:])
                        dma_rr[(2 * c + qt) % 3].dma_start(
                            d_outp[2 * c + qt], out_sb[:, o:o + 257])

                # Accumulation-group layout: concurrently open groups must live
                # on DIFFERENT PSUM tiles (same-tile interleaving loses earlier
                # partials on this stack), so each phase pairs qt0/qt1 of one g
                # (tiles s_ps[0]/s_ps[1]) wave-interleaved; tails deferred one
                # phase so PE never stalls on the exp dependency.
                mcap = max(1, 512 // WS)  # PSUM out-AP element limit per MM
                pending = []
                for g in range(G):
                    lo = goff[g]
                    for j in range(0, DEG + 1):  # waves, j ascending
                        ln = jlens[j]
                        for qt in range(2):
                            for hp in range(2):
                                for m0 in range(0, ln, mcap):
                                    mn = min(mcap, ln - m0)
                                    o = (g * NPAIR + joff[j] + m0) * WS
                                    rhs = strips[hp][:, o:o + mn * WS] \
                                        .rearrange("p (m w) -> p m w", m=mn)
                                    out = s_ps[qt][:, lo:lo + WS].unsqueeze(1) \
                                        .broadcast_to((128, mn, WS))
                                    nc.tensor.matmul(
                                        out,
                                        QP[g][j][hp][:, qt * 128:(qt + 1) * 128],
                                        rhs,
                                        start=(j == 0 and hp == 0 and m0 == 0),
                                        stop=(j == DEG and hp == 1
                                              and m0 + mn == ln))
                    for p in pending:
                        tail(*p)
                    pending = [(0, g), (1, g)]
                for p in pending:
                    tail(*p)
    nc.compile()
    return nc


_NCS = {}


def _get_nc(gsizes):
    gsizes = tuple(gsizes)
    if gsizes not in _NCS:
        _NCS[gsizes] = build_nc(gsizes)
    return _NCS[gsizes]


def _plan(valid_lens):
    """Deal valid-key chunks into a UNIFORM per-core group structure.

    Every core gets group slots of sizes (ceil(n/2), floor(n/2)) chunks, each
    slot single-batch (padded with dummy chunks where needed), so one compiled
    kernel variant serves all 8 cores in a single SPMD launch.

    Returns (core_plans, nchunks): core_plans[i] = (chunks, groups) with
    chunks = [(b, k0)] in packed order, groups = [(qbatch, slot_size)].
    """
    runs = []
    for b in range(B):
        vl = min(max(int(valid_lens[b]), 0), K)
        ks = list(range(0, vl, KC))
        if ks:
            runs.append([b, ks])
    total = sum(len(ks) for _, ks in runs)
    runs.sort(key=lambda r: -len(r[1]))
    # 16 uniform slots (2 per core) of s chunks each; min s fitting all runs
    s = max(1, -(-total // (2 * N_CORES)))
    while sum(-(-len(ks) // s) for _, ks in runs) > 2 * N_CORES:
        s += 1
    nchunks = 2 * s
    slots = []  # (batch, [k0...])
    for b, ks in runs:
        for pos in range(0, len(ks), s):
            slots.append((b, ks[pos:pos + s]))
    while len(slots) < 2 * N_CORES:
        slots.append((0, []))  # all-dummy slot (batch 0 for the q side)
    core_plans = []
    for i in range(N_CORES):
        groups, chunks = [], []
        for _ in range(2):
            b, real = slots.pop()
            groups.append((b, s))
            chunks.extend((b, k0) for k0 in real)
            chunks.extend((-1, 0) for _ in range(s - len(real)))
        core_plans.append((chunks, groups))
    return core_plans, nchunks


def kernel(queries, keys, values, valid_lens, W_q, W_k, w_v):
    import ml_dtypes
    bf16 = ml_dtypes.bfloat16
    queries = np.asarray(queries, dtype=np.float32)
    keys = np.asarray(keys, dtype=np.float32)
    values = np.asarray(values, dtype=np.float32)
    valid_lens = np.asarray(valid_lens)
    W_q = np.asarray(W_q, dtype=np.float32)
    W_k = np.asarray(W_k, dtype=np.float32)
    w_v = np.asarray(w_v, dtype=np.float32).reshape(H)

    core_plans, nchunks = _plan(valid_lens)
    W = KC * nchunks
    VCB = -(-nchunks // 4)

    wqb = W_q.astype(bf16)   # [D, H]
    wkb = W_k.astype(bf16)
    wvb = w_v.astype(bf16)
    ident = np.eye(128, dtype=np.float32)
    qTb = np.ascontiguousarray(np.transpose(queries, (0, 2, 1))).astype(bf16)
    kTb = np.ascontiguousarray(np.transpose(keys, (0, 2, 1))).astype(bf16)
    vb = values.astype(bf16)

    in_maps = []
    gsizes_per_core = []
    for cidx in range(N_CORES):
        chunks, groups = core_plans[cidx]
        gsizes = tuple(s for _, s in groups)
        gsizes_per_core.append(gsizes)
        G = len(gsizes)
        QT_OFF = 0
        KT_OFF = QT_OFF + G * 512
        WQ_OFF = KT_OFF + 2 * W
        WK_OFF = WQ_OFF + 512
        QP0_OFF = WK_OFF + 512
        V_OFF = QP0_OFF + 512
        NBF = V_OFF + VCB * 257
        NF = W + 128

        in_bf = np.zeros((128, NBF), dtype=bf16)
        in_f = np.zeros((128, NF), dtype=np.float32)
        maskrow = np.full(W, -1.0e6, dtype=np.float32)
        for g, (gb, _) in enumerate(groups):
            for dp in range(2):
                in_bf[:, QT_OFF + (g * 2 + dp) * 256:QT_OFF + (g * 2 + dp + 1) * 256] = \
                    qTb[gb][dp * 128:(dp + 1) * 128]
        for i, (b, k0) in enumerate(chunks):
            if b < 0:
                continue
            vl = int(valid_lens[b])
            n = min(KC, vl - k0)
            kcols = kTb[b][:, k0:k0 + n]
            for dp in range(2):
                in_bf[:, KT_OFF + dp * W + i * KC:KT_OFF + dp * W + i * KC + n] = \
                    kcols[dp * 128:(dp + 1) * 128]
            maskrow[i * KC:i * KC + n] = 0.0
            r, cb = KC * (i % 4), i // 4
            in_bf[r:r + n, V_OFF + cb * 257:V_OFF + cb * 257 + 256] = vb[b][k0:k0 + n]
            in_bf[r:r + n, V_OFF + cb * 257 + 256] = 1.0
        for dp in range(2):
            for hp in range(2):
                in_bf[:, WQ_OFF + (dp * 2 + hp) * 128:WQ_OFF + (dp * 2 + hp + 1) * 128] = \
                    wqb[dp * 128:(dp + 1) * 128, hp * 128:(hp + 1) * 128]
                in_bf[:, WK_OFF + (dp * 2 + hp) * 128:WK_OFF + (dp * 2 + hp + 1) * 128] = \
                    wkb[dp * 128:(dp + 1) * 128, hp * 128:(hp + 1) * 128]
        for hp in range(2):
            in_bf[:, QP0_OFF + hp * 256:QP0_OFF + (hp + 1) * 256] = \
                np.broadcast_to(wvb[hp * 128:(hp + 1) * 128, None], (128, 256))
        in_f[:, 0:W] = maskrow[None, :]
        in_f[:, W:W + 128] = ident
        in_maps.append({"in_bf": in_bf, "in_f32": in_f})

    # compile all needed variants, then run
    for gs in set(gsizes_per_core):
        _get_nc(gs)
    if len(set(gsizes_per_core)) == 1:
        nc = _get_nc(gsizes_per_core[0])
        res = run_bass_kernel_spmd(nc, in_maps, core_ids=list(range(N_CORES)))
        results = res.results
    else:
        # run homogeneous subsets per variant
        results = [None] * N_CORES
        for gs in sorted(set(gsizes_per_core)):
            ids = [i for i in range(N_CORES) if gsizes_per_core[i] == gs]
            nc = _get_nc(gs)
            res = run_bass_kernel_spmd(nc, [in_maps[i] for i in ids], core_ids=ids)
            for i, r in zip(ids, res.results):
                results[i] = r
    return _combine(results, core_plans, values, valid_lens, nchunks)


def _combine(results, core_plans, values, valid_lens, nchunks):
    accum = np.zeros((B, Q, DV), dtype=np.float64)
    denom = np.zeros((B, Q), dtype=np.float64)
    for cidx in range(N_CORES):
        outp = results[cidx]["outp"].reshape(nchunks, 2, 128, 257)
        chunks, _ = core_plans[cidx]
        for i, (b, k0) in enumerate(chunks):
            if b < 0:
                continue
            for qt in range(2):
                accum[b, qt * 128:(qt + 1) * 128] += outp[i, qt][:, :256]
                denom[b, qt * 128:(qt + 1) * 128] += outp[i, qt][:, 256]
    out = np.zeros((B, Q, DV), dtype=np.float32)
    for b in range(B):
        if int(valid_lens[b]) <= 0:
            out[b] = np.broadcast_to(values[b].mean(0), (Q, DV))
        else:
            out[b] = (accum[b] / denom[b][:, None]).astype(np.float32)
    return out


def run_spmd_traced(queries, keys, values, valid_lens, W_q, W_k, w_v, **kwargs):
    """test harness hook: same as kernel() but returns (output, BassKernelResults)."""
    res_holder = {}
    orig = run_bass_kernel_spmd

    def wrapper(nc, in_maps, core_ids, **kw):
        r = orig(nc, in_maps, core_ids=core_ids, **kw, **kwargs)
        if "res" not in res_holder:
            res_holder["res"] = r
        else:  # multiple variants: keep the max exec time
            prev = res_holder["res"]
            if (r.exec_time_ns or 0) > (prev.exec_time_ns or 0):
                res_holder["res"] = r
        return r

    g = globals()
    g["run_bass_kernel_spmd"] = wrapper
    try:
        out = kernel(queries, keys, values, valid_lens, W_q, W_k, w_v)
    finally:
        g["run_bass_kernel_spmd"] = orig
    return out, res_holder["res"]
